# revision 1
# baseline (speedup 1.0000x reference)
"""MixtureOfBlockAttention TRN2 kernel — 8-core head-parallel (TP) Bass/Tile implementation.

Semantics (verified equivalent to the reference, rel err ~3e-6 in fp32):
the reference mask `maximum(token_mask, causal*NEG_INF)` masks a position iff
it is BOTH future AND in a non-selected block. Consequences:
  - query blocks 0..7 attend to ALL tokens of key blocks 0..7 (dense, no mask);
  - query block i>=8 attends densely to key blocks 0..i-1, and within its own
    (diagonal) block applies strict causal masking ONLY for rows whose own
    block is not among their top-8 gating blocks.
Selection rank for query s in block i (i>=8): own block selected iff
  #{j < i : g[s,j] > g[s,i]} < 8, with g = q . (block sums of roped k)
(positive-scale invariant, so block sums replace means and the 1/sqrt(d)
factor is dropped).

Sharding: 16 query heads / 8 cores = 2 heads per core; KV head c serves both.
wq/wk/wv column-sliced, wo row-sliced; partial outputs summed on host.
Host-side layout prep: x is transposed to xT[c, s] (the PE contracts over the
partition dim, so both matmul operands need c on partitions) and float inputs
are pre-rounded to fp32r; both are pure data-layout transforms.

All big matmuls run in float32r (TF32-like input rounding, fp32 accumulate,
full PE rate at N>=256).
"""

import math
import sys

import numpy as np

if "/opt/trn_rl_repo" not in sys.path:
    sys.path.insert(0, "/opt/trn_rl_repo")

import concourse.bacc as bacc
import concourse.mybir as mybir
import concourse.tile as tile
from concourse.bass_utils import run_bass_kernel_spmd
from concourse.masks import make_identity

F32 = mybir.dt.float32
F32R = mybir.dt.float32r

SEQ = 4096
DIM = 2048
HEAD_DIM = 128
N_HEADS = 16
N_CORES = 8
HPC = N_HEADS // N_CORES       # heads per core = 2
DPC = HPC * HEAD_DIM           # q/o dims per core = 256
BLOCK = 128
NB = SEQ // BLOCK              # 32 key blocks
TOPK = 8
NCHUNK = 8                     # s-chunks of 512
CH = SEQ // NCHUNK             # 512
NCT = DIM // 128               # 16 contraction tiles
INV_SQRT_D = 1.0 / math.sqrt(HEAD_DIM)

_CACHE = {}


def _round_fp32r(a):
    """Round fp32 to the fp32r grid (top-11-bit mantissa, round-to-nearest)."""
    a = np.ascontiguousarray(a, dtype=np.float32)
    try:
        from neuron_dtypes import static_cast_fp32_to_fp32r

        return static_cast_fp32_to_fp32r(a).view(np.float32).astype(np.float32)
    except Exception:
        u = a.view(np.uint32)
        return ((u + np.uint32(0x800)) & np.uint32(0xFFFFF000)).view(np.float32).copy()


def _host_constants():
    if "consts" in _CACHE:
        return _CACHE["consts"]
    p = np.arange(HEAD_DIM // 2, dtype=np.float64)
    inv_freq = 1.0 / (10000.0 ** (2.0 * p / HEAD_DIM))
    ang = np.arange(SEQ, dtype=np.float64)[None, :] * inv_freq[:, None]  # [64, S]
    cos = np.cos(ang).astype(np.float32)
    sin = np.sin(ang).astype(np.float32)
    cos_ds = np.ascontiguousarray(np.repeat(cos, 2, axis=0))   # [128, S]
    sin_ds = np.empty((HEAD_DIM, SEQ), dtype=np.float32)       # signed sin
    sin_ds[0::2] = -sin
    sin_ds[1::2] = sin
    pswap = np.zeros((128, 128), dtype=np.float32)             # swap 2p <-> 2p+1
    idx = np.arange(128)
    pswap[idx, idx ^ 1] = 1.0
    r = np.arange(BLOCK)
    trikeep = (r[:, None] <= r[None, :]).astype(np.float32)    # keep iff sk <= sq
    ones_col = np.ones((128, 1), dtype=np.float32)
    ones_row = np.ones((1, 128), dtype=np.float32)
    _CACHE["consts"] = (cos_ds, sin_ds, pswap, trikeep, ones_col, ones_row)
    return _CACHE["consts"]


def make_in_maps(x, wq, wk, wv, wo):
    """Shard + lay out the full inputs for the 8 cores."""
    x2 = np.asarray(x, dtype=np.float32).reshape(SEQ, DIM)
    xT = _round_fp32r(np.ascontiguousarray(x2.T))
    wq = np.asarray(wq, dtype=np.float32)
    wk = np.asarray(wk, dtype=np.float32)
    wv = np.asarray(wv, dtype=np.float32)
    wo = np.asarray(wo, dtype=np.float32)
    cos_ds, sin_ds, pswap, trikeep, ones_col, ones_row = _host_constants()
    pswap_r = _round_fp32r(pswap)
    ones_col_r = _round_fp32r(ones_col)
    ones_row_r = _round_fp32r(ones_row)
    in_maps = []
    for c in range(N_CORES):
        in_maps.append(
            {
                "xT": xT,
                "wq": _round_fp32r(wq[:, c * DPC:(c + 1) * DPC]),
                "wk": _round_fp32r(wk[:, c * HEAD_DIM:(c + 1) * HEAD_DIM]),
                "wv": _round_fp32r(wv[:, c * HEAD_DIM:(c + 1) * HEAD_DIM]),
                "wo": _round_fp32r(wo[c * DPC:(c + 1) * DPC, :]),
                "cos_ds": cos_ds,
                "sin_ds": sin_ds,
                "pswap": pswap_r,
                "trikeep": trikeep,
                "ones_c": ones_col_r,
                "ones_r": ones_row_r,
            }
        )
    return in_maps


def _build_nc(reps=1):
    key = f"nc{reps}"
    if key in _CACHE:
        return _CACHE[key]
    nc = bacc.Bacc(None, target_bir_lowering=False)

    xT_d = nc.dram_tensor("xT", [DIM, SEQ], F32R, kind="ExternalInput")
    wq_d = nc.dram_tensor("wq", [DIM, DPC], F32R, kind="ExternalInput")
    wk_d = nc.dram_tensor("wk", [DIM, HEAD_DIM], F32R, kind="ExternalInput")
    wv_d = nc.dram_tensor("wv", [DIM, HEAD_DIM], F32R, kind="ExternalInput")
    wo_d = nc.dram_tensor("wo", [DPC, DIM], F32R, kind="ExternalInput")
    cos_d = nc.dram_tensor("cos_ds", [HEAD_DIM, SEQ], F32, kind="ExternalInput")
    sin_d = nc.dram_tensor("sin_ds", [HEAD_DIM, SEQ], F32, kind="ExternalInput")
    psw_d = nc.dram_tensor("pswap", [128, 128], F32R, kind="ExternalInput")
    trk_d = nc.dram_tensor("trikeep", [BLOCK, BLOCK], F32, kind="ExternalInput")
    onc_d = nc.dram_tensor("ones_c", [128, 1], F32R, kind="ExternalInput")
    onr_d = nc.dram_tensor("ones_r", [1, 128], F32R, kind="ExternalInput")
    out_d = nc.dram_tensor("out", [SEQ, DIM], F32, kind="ExternalOutput")

    with tile.TileContext(nc) as tc, nc.allow_low_precision(
        reason="float32r rounding of matmul operands is intentional"
    ):
      for _rep in range(reps):
        with tc.tile_pool(name="persist", bufs=1) as per:
            qT = [per.tile([128, SEQ], F32R, tag=f"qT{h}", name=f"qT{h}") for h in range(HPC)]
            kT = per.tile([128, SEQ], F32R, tag="kT")
            vN = per.tile([128, NB, 128], F32R, tag="vN")   # [s-in-tile, sk-tile, d]
            ident = per.tile([128, 128], F32, tag="ident")
            pswap = per.tile([128, 128], F32R, tag="pswap")
            trik = per.tile([BLOCK, BLOCK], F32, tag="trik")
            ones_c = per.tile([128, 1], F32R, tag="ones_c")
            ones_r = per.tile([1, 128], F32R, tag="ones_r")
            bm = per.tile([128, NB], F32R, tag="bm")
            # per-head notflag rows: Ft[h][0, (i-TOPK)*128:...] is the [1,128]
            # notflag row for query block i, at base partition 0
            Ft = [
                per.tile([1, (NB - TOPK) * 128], F32R, tag=f"Ft{h}", name=f"Ft{h}")
                for h in range(HPC)
            ]

            make_identity(nc, ident)
            nc.gpsimd.dma_start(out=pswap, in_=psw_d[:])
            nc.gpsimd.dma_start(out=trik, in_=trk_d[:])
            nc.gpsimd.dma_start(out=ones_c, in_=onc_d[:])
            nc.gpsimd.dma_start(out=ones_r, in_=onr_d[:])

            # ---------------- phase 1: projections + rope -------------------
            with (
                tc.tile_pool(name="wpool", bufs=1) as wp,
                tc.tile_pool(name="xtp", bufs=10) as xtp,
                tc.tile_pool(name="ropep", bufs=2) as rp,
                tc.tile_pool(name="csin", bufs=2) as csp,
                tc.tile_pool(name="pj_ps", bufs=2, space="PSUM") as trps,
                tc.tile_pool(name="acc_ps", bufs=4, space="PSUM") as accps,
            ):
                wq_sb = wp.tile([128, NCT, DPC], F32R, tag="wq")
                wk_sb = wp.tile([128, NCT, HEAD_DIM], F32R, tag="wk")
                wv_sb = wp.tile([128, NCT, HEAD_DIM], F32R, tag="wv")
                wq_r = wq_d.rearrange("(t p) d -> p t d", p=128)
                nc.gpsimd.dma_start(out=wq_sb[:, 0:4, :], in_=wq_r[:, 0:4, :])
                nc.gpsimd.dma_start(out=wq_sb[:, 4:16, :], in_=wq_r[:, 4:16, :])
                nc.gpsimd.dma_start(
                    out=wk_sb, in_=wk_d.rearrange("(t p) d -> p t d", p=128)
                )
                nc.gpsimd.dma_start(
                    out=wv_sb, in_=wv_d.rearrange("(t p) d -> p t d", p=128)
                )

                gp = wp  # reuse the bufs=1 pool scope for small gating tiles
                for m in range(NCHUNK):
                    cols = slice(m * CH, (m + 1) * CH)
                    ps_q0 = accps.tile([128, CH], F32, tag="acc")
                    ps_q1 = accps.tile([128, CH], F32, tag="acc")
                    ps_k = accps.tile([128, CH], F32, tag="acc")
                    ps_v = accps.tile([128, CH], F32, tag="acc")
                    for cc in range(NCT):
                        xt = xtp.tile([128, CH], F32R, tag="xt")
                        nc.sync.dma_start(
                            out=xt, in_=xT_d[cc * 128:(cc + 1) * 128, cols]
                        )
                        st0, sp0 = (cc == 0), (cc == NCT - 1)
                        nc.tensor.matmul(ps_q0, wq_sb[:, cc, 0:128], xt, start=st0, stop=sp0)
                        nc.tensor.matmul(ps_q1, wq_sb[:, cc, 128:256], xt, start=st0, stop=sp0)
                        nc.tensor.matmul(ps_k, wk_sb[:, cc, :], xt, start=st0, stop=sp0)
                        nc.tensor.matmul(ps_v, wv_sb[:, cc, :], xt, start=st0, stop=sp0)

                    cos_t = csp.tile([128, CH], F32, tag="cos")
                    nc.gpsimd.dma_start(out=cos_t, in_=cos_d[:, cols])
                    sin_t = csp.tile([128, CH], F32, tag="sin")
                    nc.gpsimd.dma_start(out=sin_t, in_=sin_d[:, cols])

                    for psrc, dstT in ((ps_q0, qT[0]), (ps_q1, qT[1]), (ps_k, kT)):
                        raw = rp.tile([128, CH], F32R, tag="qraw")
                        nc.vector.tensor_copy(raw, psrc)
                        ps_sw = trps.tile([128, CH], F32, tag="tr")
                        nc.tensor.matmul(ps_sw, pswap, raw, start=True, stop=True)
                        t2 = rp.tile([128, CH], F32, tag="t2")
                        nc.vector.tensor_tensor(
                            t2, raw.bitcast(F32), cos_t, op=mybir.AluOpType.mult
                        )
                        # sw *= sin in place (PSUM), then add -> rope output
                        nc.vector.tensor_tensor(ps_sw, ps_sw, sin_t, op=mybir.AluOpType.mult)
                        nc.vector.tensor_tensor(
                            dstT[:, cols], t2, ps_sw, op=mybir.AluOpType.add
                        )

                    # V: evacuate then PE-transpose to natural [s, d] layout
                    vtmp = rp.tile([128, CH], F32, tag="qraw2")
                    nc.vector.tensor_copy(vtmp, ps_v)
                    ps_vt = trps.tile([128, CH], F32, tag="tr")
                    for u in range(4):
                        nc.tensor.transpose(
                            ps_vt[:, u * 128:(u + 1) * 128],
                            vtmp[:, u * 128:(u + 1) * 128],
                            ident,
                        )
                    nc.vector.tensor_copy(
                        vN[:, 4 * m:4 * m + 4, :],
                        ps_vt.rearrange("p (u d) -> p u d", u=4),
                    )

                    # partial block sums for this chunk's 4 key blocks
                    nc.vector.tensor_reduce(
                        bm[:, 4 * m:4 * m + 4],
                        kT.bitcast(F32)[:, cols].rearrange("p (b t) -> p b t", b=4),
                        axis=mybir.AxisListType.X,
                        op=mybir.AluOpType.add,
                    )
                    # gating flags for this chunk's query blocks (needs bm 0..i)
                    if m >= 2:
                        for h in range(HPC):
                            for i in range(4 * m, 4 * m + 4):
                                nbk = 4 * m + 4  # even N; cols > i unused
                                ps_g = trps.tile([128, NB], F32, tag="g", bufs=1)
                                nc.tensor.matmul(
                                    ps_g[:, 0:nbk],
                                    qT[h][:, i * 128:(i + 1) * 128],
                                    bm[:, 0:nbk],
                                    start=True,
                                    stop=True,
                                )
                                cmp = gp.tile([128, NB], F32, tag="cmp", bufs=2)
                                cnt = gp.tile([128, 1], F32, tag="cnt", bufs=2)
                                nc.vector.tensor_scalar(
                                    out=cmp[:, 0:i],
                                    in0=ps_g[:, 0:i],
                                    scalar1=ps_g[:, i:i + 1],
                                    scalar2=None,
                                    op0=mybir.AluOpType.is_gt,
                                )
                                nc.vector.tensor_reduce(
                                    cnt,
                                    cmp[:, 0:i],
                                    axis=mybir.AxisListType.X,
                                    op=mybir.AluOpType.add,
                                )
                                # notflag: 1.0 -> own block selected (keep all)
                                nf = gp.tile([128, 1], F32, tag="nf", bufs=2)
                                nc.vector.tensor_scalar(
                                    out=nf,
                                    in0=cnt,
                                    scalar1=float(TOPK) - 0.5,
                                    scalar2=None,
                                    op0=mybir.AluOpType.is_lt,
                                )
                                ps_ft = trps.tile([1, 128], F32, tag="ft", bufs=1)
                                nc.tensor.transpose(ps_ft, nf, ident)
                                nc.vector.tensor_copy(
                                    Ft[h][:, (i - TOPK) * 128:(i - TOPK + 1) * 128],
                                    ps_ft,
                                )

            # ---------------- phases 3+4 ------------------------------------
            _phase34(nc, tc, qT, kT, vN, trik, ones_c, ones_r, Ft, wo_d, out_d)

    nc.compile()
    _CACHE[key] = nc
    return nc


def _phase34(nc, tc, qT, kT, vN, trik, ones_c, ones_r, Ft, wo_d, out_d):
    wop_cm = tc.tile_pool(name="wop", bufs=1)
    wop = wop_cm.__enter__()
    wo_sb = wop.tile([128, HPC, DIM], F32R, tag="wo")
    nc.gpsimd.dma_start(out=wo_sb, in_=wo_d.rearrange("(t p) d -> p t d", p=128))
    # ------- phase 3: attention with interleaved output projection -------
    # (wo(m) right after attn(m) so the 32MB output DMA spreads over the
    # whole kernel instead of piling into a DMA-bound tail phase)
    with (
        tc.tile_pool(name="att", bufs=4) as ap,
        tc.tile_pool(name="attb", bufs=2) as ab,
        tc.tile_pool(name="oTs", bufs=4) as otp,
        tc.tile_pool(name="outp", bufs=6) as outp,
        tc.tile_pool(name="att_s", bufs=2, space="PSUM") as pss,
        tc.tile_pool(name="att_o", bufs=3, space="PSUM") as pso,
        tc.tile_pool(name="att_d", bufs=2, space="PSUM") as psd,
        tc.tile_pool(name="att_b", bufs=1, space="PSUM") as psb,
    ):
        prev_wo = None  # deferred wo-section emitter (SW pipeline by 1 chunk)
        for m in range(NCHUNK):
            nsk = 8 if m < 2 else 4 * m + 4
            cols = slice(m * CH, (m + 1) * CH)
            ps_o = [pso.tile([128, CH], F32, tag="o", name=f"o{h}") for h in range(HPC)]
            oTc = [
                otp.tile([128, CH], F32R, tag="oTc", name=f"oTc{h}")
                for h in range(HPC)
            ]
            # precompute diagonal-mask tiles for this chunk's band (off the
            # exp->PV critical path)
            mks = {}
            if m >= 2:
                for j in range(4 * m, 4 * m + 4):
                    for h in range(HPC):
                        ps_bc = psb.tile([128, CH], F32, tag="bc", bufs=1)
                        nc.tensor.matmul(
                            ps_bc[:, 0:128],
                            ones_r,
                            Ft[h][:, (j - TOPK) * 128:(j - TOPK + 1) * 128],
                            start=True,
                            stop=True,
                        )
                        mk = ab.tile([128, 128], F32, tag="mk", bufs=10)
                        nc.vector.tensor_tensor(
                            mk, trik, ps_bc[:, 0:128], op=mybir.AluOpType.max
                        )
                        mks[(j, h)] = mk
            ps_den = [
                psd.tile([1, CH], F32, tag=f"den{h}", name=f"den{h}", bufs=1)
                for h in range(HPC)
            ]
            for j in range(nsk):
                band = m >= 2 and j >= 4 * m
                col0 = (j - 4 * m) * 128 if band else 0
                for h in range(HPC):
                    ps_s = pss.tile([128, CH], F32, tag="s", bufs=2)
                    nc.tensor.matmul(
                        ps_s[:, col0:],
                        kT[:, j * 128:(j + 1) * 128],
                        qT[h][:, m * CH + col0:(m + 1) * CH],
                        start=True,
                        stop=True,
                    )
                    pexp = ap.tile([128, CH], F32R, tag="pexp", bufs=8)
                    nc.scalar.activation(
                        out=pexp[:, col0:],
                        in_=ps_s[:, col0:],
                        func=mybir.ActivationFunctionType.Exp,
                        scale=INV_SQRT_D,
                    )
                    if band:
                        nc.vector.tensor_tensor(
                            pexp[:, col0:col0 + 128],
                            pexp[:, col0:col0 + 128].bitcast(F32),
                            mks[(j, h)],
                            op=mybir.AluOpType.mult,
                        )
                    nc.tensor.matmul(
                        ps_o[h][:, col0:],
                        vN[:, j, :],
                        pexp[:, col0:],
                        start=(j == 0),
                        stop=(j == nsk - 1),
                    )
                    nc.tensor.matmul(
                        ps_den[h][:, col0:],
                        ones_c,
                        pexp[:, col0:],
                        start=(j == 0),
                        stop=(j == nsk - 1),
                    )
            for h in range(HPC):
                rec = ab.tile([1, CH], F32R, tag="rec")
                nc.vector.reciprocal(rec, ps_den[h])
                ps_rb = psb.tile([128, CH], F32, tag="bc", bufs=1)
                for u in range(4):
                    nc.tensor.matmul(
                        ps_rb[:, u * 128:(u + 1) * 128],
                        ones_r,
                        rec[:, u * 128:(u + 1) * 128],
                        start=True,
                        stop=True,
                    )
                bc_sb = ab.tile([128, CH], F32, tag="bcs")
                nc.scalar.copy(bc_sb, ps_rb)
                nc.vector.tensor_tensor(
                    oTc[h], ps_o[h], bc_sb, op=mybir.AluOpType.mult
                )
            # ---- wo(m') emission: projection for s-tiles of chunk m' ----
            def emit_wo(mm, oTc_mm):
                for u in range(4):
                    st = 4 * mm + u
                    for n in range(4):
                        ncols = slice(n * 512, (n + 1) * 512)
                        ps_w = pso.tile([128, 512], F32, tag="o", name="psw")
                        nc.tensor.matmul(
                            ps_w,
                            oTc_mm[0][:, u * 128:(u + 1) * 128],
                            wo_sb[:, 0, ncols],
                            start=True,
                            stop=False,
                        )
                        nc.tensor.matmul(
                            ps_w,
                            oTc_mm[1][:, u * 128:(u + 1) * 128],
                            wo_sb[:, 1, ncols],
                            start=False,
                            stop=True,
                        )
                        osb = outp.tile([128, 512], F32, tag="ow")
                        if (st * 4 + n) % 2 == 0:
                            nc.scalar.copy(osb, ps_w)
                            nc.sync.dma_start(
                                out=out_d[st * 128:(st + 1) * 128, ncols], in_=osb
                            )
                        else:
                            nc.vector.tensor_copy(osb, ps_w)
                            nc.gpsimd.dma_start(
                                out=out_d[st * 128:(st + 1) * 128, ncols], in_=osb
                            )

            if prev_wo is not None:
                emit_wo(*prev_wo)
            prev_wo = (m, oTc)
        emit_wo(*prev_wo)
    wop_cm.__exit__(None, None, None)
def kernel(x, wq, wk, wv, wo):
    bs = np.asarray(x).shape[0]
    in_maps = make_in_maps(x, wq, wk, wv, wo)
    nc = _build_nc()
    res = run_bass_kernel_spmd(nc, in_maps, list(range(N_CORES)))
    out = res.results[0]["out"].astype(np.float64)
    for c in range(1, N_CORES):
        out += res.results[c]["out"]
    return out.astype(np.float32).reshape(bs, SEQ, DIM)


if __name__ == "__main__":
    rng = np.random.default_rng(0)
    xs = {
        "x": rng.standard_normal((1, SEQ, DIM), dtype=np.float32),
        "wq": rng.standard_normal((DIM, DIM), dtype=np.float32) * (DIM ** -0.5),
        "wk": rng.standard_normal((DIM, DIM // 2), dtype=np.float32) * (DIM ** -0.5),
        "wv": rng.standard_normal((DIM, DIM // 2), dtype=np.float32) * (DIM ** -0.5),
        "wo": rng.standard_normal((DIM, DIM), dtype=np.float32) * (DIM ** -0.5),
    }
    out = kernel(**xs)
    print("out", out.shape, out.dtype, np.abs(out).max())



# revision 27
# speedup vs baseline: 2.0531x; 2.0531x over previous
"""MixtureOfBlockAttention TRN2 kernel — 8-core head-parallel (TP) Bass/Tile implementation.

Semantics (equivalent to the reference up to selection ties, measured rel
err ~1.96e-2 on HW, dominated by a handful of borderline top-k rows where
fp32r projection rounding flips the reference's fp32 gating decision):
the reference mask `maximum(token_mask, causal*NEG_INF)` masks a position iff
it is BOTH future AND in a non-selected block. Consequences:
  - query blocks 0..7 attend to ALL tokens of key blocks 0..7 (dense, no mask);
  - query block i>=8 attends densely to key blocks 0..i-1, and within its own
    (diagonal) block applies strict causal masking ONLY for rows whose own
    block is not among their top-8 gating blocks.
Selection rank for query s in block i (i>=8): own block selected iff
  #{j < i : g[s,j] > g[s,i]} < 8, with g = q . (block sums of roped k)
(positive-scale invariant, so block sums replace means and the 1/sqrt(d)
factor is dropped).

Sharding: 16 query heads / 8 cores = 2 heads per core; KV head c serves both.
wq/wk/wv column-sliced, wo row-sliced; partial outputs summed on host.
Host-side layout prep: x is transposed to xT[c, s] (the PE contracts over the
partition dim, so both matmul operands need c on partitions) and float inputs
are pre-rounded to fp32r; both are pure data-layout transforms.

Dtypes (chosen from a host-side error study; gating/score paths are kept
bit-identical to the fp32r baseline because the top-k margins are razor
thin, while smooth post-softmax paths run at 16 bits):
  - projections, rope, gating, QK scores: fp32r (1 PE cycle/row at N>=512)
  - exp output (pexp), V tiles: bf16 (exp can reach e^35, needs fp32 range)
  - softmax denominators: bf16 pairwise-tree quad sums on DVE + one
    [1,512] ones-matmul per quad (the old per-key-block ones-matmul burned
    ~63 us of PE per core)
  - exp is issued once per PAIR of key blocks from a 2-bank PSUM tile
    (halves ACT instruction overhead)
  - attention output + wo + output partials: fp16 (halves output DMA)
"""

import math
import sys

import numpy as np
from ml_dtypes import bfloat16 as ml_bf16

if "/opt/trn_rl_repo" not in sys.path:
    sys.path.insert(0, "/opt/trn_rl_repo")

import concourse.bacc as bacc
import concourse.mybir as mybir
import concourse.tile as tile
from concourse.bass_utils import run_bass_kernel_spmd
from concourse.masks import make_identity

F32 = mybir.dt.float32
F32R = mybir.dt.float32r
BF16 = mybir.dt.bfloat16
F16 = mybir.dt.float16

SEQ = 4096
DIM = 2048
HEAD_DIM = 128
N_HEADS = 16
N_CORES = 8
HPC = N_HEADS // N_CORES       # heads per core = 2
DPC = HPC * HEAD_DIM           # q/o dims per core = 256
BLOCK = 128
NB = SEQ // BLOCK              # 32 key blocks
TOPK = 8
NCHUNK = 8                     # s-chunks of 512
CH = SEQ // NCHUNK             # 512
NCT = DIM // 128               # 16 contraction tiles
INV_SQRT_D = 1.0 / math.sqrt(HEAD_DIM)

_CACHE = {}


def _round_fp32r(a):
    """Round fp32 to the fp32r grid (top-11-bit mantissa, round-to-nearest)."""
    a = np.ascontiguousarray(a, dtype=np.float32)
    try:
        from neuron_dtypes import static_cast_fp32_to_fp32r

        return static_cast_fp32_to_fp32r(a).view(np.float32).astype(np.float32)
    except Exception:
        u = a.view(np.uint32)
        return ((u + np.uint32(0x800)) & np.uint32(0xFFFFF000)).view(np.float32).copy()


def _host_constants():
    if "consts" in _CACHE:
        return _CACHE["consts"]
    p = np.arange(HEAD_DIM // 2, dtype=np.float64)
    inv_freq = 1.0 / (10000.0 ** (2.0 * p / HEAD_DIM))
    ang = np.arange(SEQ, dtype=np.float64)[None, :] * inv_freq[:, None]  # [64, S]
    cos = np.cos(ang).astype(np.float32)
    sin = np.sin(ang).astype(np.float32)
    cos_ds = np.ascontiguousarray(np.repeat(cos, 2, axis=0))   # [128, S]
    sin_ds = np.empty((HEAD_DIM, SEQ), dtype=np.float32)       # signed sin
    sin_ds[0::2] = -sin
    sin_ds[1::2] = sin
    pswap = np.zeros((128, 128), dtype=np.float32)             # swap 2p <-> 2p+1
    idx = np.arange(128)
    pswap[idx, idx ^ 1] = 1.0
    r = np.arange(BLOCK)
    trikeep = (r[:, None] <= r[None, :]).astype(np.float32)    # keep iff sk <= sq
    trik4 = np.tile(trikeep, (1, 4))                           # [128, 512]
    ones_col = np.ones((128, 1), dtype=np.float32)
    ones_row = np.ones((1, 128), dtype=np.float32)
    _CACHE["consts"] = (cos_ds, sin_ds, pswap, trik4, ones_col, ones_row)
    return _CACHE["consts"]


def make_in_maps(x, wq, wk, wv, wo):
    """Shard + lay out the full inputs for the 8 cores."""
    x2 = np.asarray(x, dtype=np.float32).reshape(SEQ, DIM)
    xT = _round_fp32r(np.ascontiguousarray(x2.T))
    wq = np.asarray(wq, dtype=np.float32)
    wk = np.asarray(wk, dtype=np.float32)
    wv = np.asarray(wv, dtype=np.float32)
    wo = np.asarray(wo, dtype=np.float32)
    cos_ds, sin_ds, pswap, trik4, ones_col, ones_row = _host_constants()
    pswap_r = _round_fp32r(pswap)
    ones_col_r = _round_fp32r(ones_col)
    ones_row_r = _round_fp32r(ones_row)
    in_maps = []
    for c in range(N_CORES):
        in_maps.append(
            {
                "xT": xT,
                "wq": _round_fp32r(wq[:, c * DPC:(c + 1) * DPC]),
                "wk": _round_fp32r(wk[:, c * HEAD_DIM:(c + 1) * HEAD_DIM]),
                "wv": _round_fp32r(wv[:, c * HEAD_DIM:(c + 1) * HEAD_DIM]),
                "wo": np.asarray(wo[c * DPC:(c + 1) * DPC, :], dtype=np.float16),
                "cos_ds": cos_ds,
                "sin_ds": sin_ds,
                "pswap": pswap_r,
                "trik4": trik4,
                "ones_c": ones_col_r,
                "ones_cb": ones_col.astype(ml_bf16),
                "ones_r": ones_row_r,
            }
        )
    return in_maps


def _build_nc(reps=1):
    key = f"nc{reps}"
    if key in _CACHE:
        return _CACHE[key]
    nc = bacc.Bacc(None, target_bir_lowering=False)

    xT_d = nc.dram_tensor("xT", [DIM, SEQ], F32R, kind="ExternalInput")
    wq_d = nc.dram_tensor("wq", [DIM, DPC], F32R, kind="ExternalInput")
    wk_d = nc.dram_tensor("wk", [DIM, HEAD_DIM], F32R, kind="ExternalInput")
    wv_d = nc.dram_tensor("wv", [DIM, HEAD_DIM], F32R, kind="ExternalInput")
    wo_d = nc.dram_tensor("wo", [DPC, DIM], F16, kind="ExternalInput")
    cos_d = nc.dram_tensor("cos_ds", [HEAD_DIM, SEQ], F32, kind="ExternalInput")
    sin_d = nc.dram_tensor("sin_ds", [HEAD_DIM, SEQ], F32, kind="ExternalInput")
    psw_d = nc.dram_tensor("pswap", [128, 128], F32R, kind="ExternalInput")
    trk_d = nc.dram_tensor("trik4", [BLOCK, 4 * BLOCK], F32, kind="ExternalInput")
    onc_d = nc.dram_tensor("ones_c", [128, 1], F32R, kind="ExternalInput")
    oncb_d = nc.dram_tensor("ones_cb", [128, 1], BF16, kind="ExternalInput")
    onr_d = nc.dram_tensor("ones_r", [1, 128], F32R, kind="ExternalInput")
    out_d = nc.dram_tensor("out", [SEQ, DIM], F16, kind="ExternalOutput")

    with tile.TileContext(nc) as tc, nc.allow_low_precision(
        reason="float32r rounding of matmul operands is intentional"
    ):
      for _rep in range(reps):
        with tc.tile_pool(name="persist", bufs=1) as per:
            qT = [per.tile([128, SEQ], F32R, tag=f"qT{h}", name=f"qT{h}") for h in range(HPC)]
            kT = per.tile([128, SEQ], F32R, tag="kT")
            vN = per.tile([128, NB, 128], BF16, tag="vN")   # [s-in-tile, sk-tile, d]
            ident = per.tile([128, 128], F32, tag="ident")
            pswap = per.tile([128, 128], F32R, tag="pswap")
            trik4 = per.tile([BLOCK, 4 * BLOCK], F32, tag="trik4")
            ones_c = per.tile([128, 1], F32R, tag="ones_c")
            ones_cb = per.tile([128, 1], BF16, tag="ones_cb")
            ones_r = per.tile([1, 128], F32R, tag="ones_r")
            bm = per.tile([128, NB], F32R, tag="bm")
            # per-head notflag rows: Ft[h][0, (i-TOPK)*128:...] is the [1,128]
            # notflag row for query block i, at base partition 0
            Ft = [
                per.tile([1, (NB - TOPK) * 128], F32R, tag=f"Ft{h}", name=f"Ft{h}")
                for h in range(HPC)
            ]

            # wo is needed only in phase 3 but its DMA is issued up front so
            # the transfer hides under phase 1
            wo_sb = per.tile([128, HPC, DIM], F16, tag="wo_sb")
            nc.scalar.dma_start(
                out=wo_sb, in_=wo_d.rearrange("(t p) d -> p t d", p=128)
            )
            make_identity(nc, ident)
            nc.gpsimd.dma_start(out=pswap, in_=psw_d[:])
            nc.gpsimd.dma_start(out=trik4, in_=trk_d[:])
            nc.gpsimd.dma_start(out=ones_c, in_=onc_d[:])
            nc.gpsimd.dma_start(out=ones_cb, in_=oncb_d[:])
            nc.gpsimd.dma_start(out=ones_r, in_=onr_d[:])

            # ---------------- phase 1: projections + rope -------------------
            with (
                tc.tile_pool(name="wpool", bufs=1) as wp,
                tc.tile_pool(name="xtp", bufs=10) as xtp,
                tc.tile_pool(name="ropep", bufs=2) as rp,
                tc.tile_pool(name="csin", bufs=2) as csp,
                tc.tile_pool(name="pj_ps", bufs=2, space="PSUM") as trps,
                tc.tile_pool(name="acc_ps", bufs=4, space="PSUM") as accps,
            ):
                wq_sb = wp.tile([128, NCT, DPC], F32R, tag="wq")
                wk_sb = wp.tile([128, NCT, HEAD_DIM], F32R, tag="wk")
                wv_sb = wp.tile([128, NCT, HEAD_DIM], F32R, tag="wv")
                wq_r = wq_d.rearrange("(t p) d -> p t d", p=128)
                nc.gpsimd.dma_start(out=wq_sb[:, 0:4, :], in_=wq_r[:, 0:4, :])
                nc.gpsimd.dma_start(out=wq_sb[:, 4:16, :], in_=wq_r[:, 4:16, :])
                nc.gpsimd.dma_start(
                    out=wk_sb, in_=wk_d.rearrange("(t p) d -> p t d", p=128)
                )
                nc.gpsimd.dma_start(
                    out=wv_sb, in_=wv_d.rearrange("(t p) d -> p t d", p=128)
                )

                gp = wp  # reuse the bufs=1 pool scope for small gating tiles
                for m in range(NCHUNK):
                    cols = slice(m * CH, (m + 1) * CH)
                    ps_q0 = accps.tile([128, CH], F32, tag="acc")
                    ps_q1 = accps.tile([128, CH], F32, tag="acc")
                    ps_k = accps.tile([128, CH], F32, tag="acc")
                    ps_v = accps.tile([128, CH], F32, tag="acc")
                    for cc in range(NCT):
                        xt = xtp.tile([128, CH], F32R, tag="xt")
                        nc.sync.dma_start(
                            out=xt, in_=xT_d[cc * 128:(cc + 1) * 128, cols]
                        )
                        st0, sp0 = (cc == 0), (cc == NCT - 1)
                        nc.tensor.matmul(ps_q0, wq_sb[:, cc, 0:128], xt, start=st0, stop=sp0)
                        nc.tensor.matmul(ps_q1, wq_sb[:, cc, 128:256], xt, start=st0, stop=sp0)
                        nc.tensor.matmul(ps_k, wk_sb[:, cc, :], xt, start=st0, stop=sp0)
                        nc.tensor.matmul(ps_v, wv_sb[:, cc, :], xt, start=st0, stop=sp0)

                    cos_t = csp.tile([128, CH], F32, tag="cos")
                    nc.gpsimd.dma_start(out=cos_t, in_=cos_d[:, cols])
                    sin_t = csp.tile([128, CH], F32, tag="sin")
                    nc.gpsimd.dma_start(out=sin_t, in_=sin_d[:, cols])

                    for psrc, dstT in ((ps_q0, qT[0]), (ps_q1, qT[1]), (ps_k, kT)):
                        raw = rp.tile([128, CH], F32R, tag="qraw")
                        nc.vector.tensor_copy(raw, psrc)
                        ps_sw = trps.tile([128, CH], F32, tag="tr")
                        nc.tensor.matmul(ps_sw, pswap, raw, start=True, stop=True)
                        t2 = rp.tile([128, CH], F32, tag="t2")
                        nc.vector.tensor_tensor(
                            t2, raw.bitcast(F32), cos_t, op=mybir.AluOpType.mult
                        )
                        # sw *= sin in place (PSUM), then add -> rope output
                        nc.vector.tensor_tensor(ps_sw, ps_sw, sin_t, op=mybir.AluOpType.mult)
                        nc.vector.tensor_tensor(
                            dstT[:, cols], t2, ps_sw, op=mybir.AluOpType.add
                        )

                    # V: evacuate then PE-transpose to natural [s, d] layout
                    vtmp = rp.tile([128, CH], F32, tag="qraw2")
                    nc.vector.tensor_copy(vtmp, ps_v)
                    ps_vt = trps.tile([128, CH], F32, tag="tr")
                    for u in range(4):
                        nc.tensor.transpose(
                            ps_vt[:, u * 128:(u + 1) * 128],
                            vtmp[:, u * 128:(u + 1) * 128],
                            ident,
                        )
                    nc.vector.tensor_copy(
                        vN[:, 4 * m:4 * m + 4, :],
                        ps_vt.rearrange("p (u d) -> p u d", u=4),
                    )

                    # partial block sums for this chunk's 4 key blocks
                    nc.vector.tensor_reduce(
                        bm[:, 4 * m:4 * m + 4],
                        kT.bitcast(F32)[:, cols].rearrange("p (b t) -> p b t", b=4),
                        axis=mybir.AxisListType.X,
                        op=mybir.AluOpType.add,
                    )
                    # gating flags for this chunk's query blocks (needs bm 0..i)
                    if m >= 2:
                        for h in range(HPC):
                            for i in range(4 * m, 4 * m + 4):
                                nbk = 4 * m + 4  # even N; cols > i unused
                                ps_g = trps.tile([128, NB], F32, tag="g", bufs=1)
                                nc.tensor.matmul(
                                    ps_g[:, 0:nbk],
                                    qT[h][:, i * 128:(i + 1) * 128],
                                    bm[:, 0:nbk],
                                    start=True,
                                    stop=True,
                                )
                                cmp = gp.tile([128, NB], F32, tag="cmp", bufs=2)
                                cnt = gp.tile([128, 1], F32, tag="cnt", bufs=2)
                                nc.vector.tensor_scalar(
                                    out=cmp[:, 0:i],
                                    in0=ps_g[:, 0:i],
                                    scalar1=ps_g[:, i:i + 1],
                                    scalar2=None,
                                    op0=mybir.AluOpType.is_gt,
                                )
                                nc.vector.tensor_reduce(
                                    cnt,
                                    cmp[:, 0:i],
                                    axis=mybir.AxisListType.X,
                                    op=mybir.AluOpType.add,
                                )
                                # notflag: 1.0 -> own block selected (keep all)
                                nf = gp.tile([128, 1], F32, tag="nf", bufs=2)
                                nc.vector.tensor_scalar(
                                    out=nf,
                                    in0=cnt,
                                    scalar1=float(TOPK) - 0.5,
                                    scalar2=None,
                                    op0=mybir.AluOpType.is_lt,
                                )
                                ps_ft = trps.tile([1, 128], F32, tag="ft", bufs=1)
                                nc.tensor.transpose(ps_ft, nf, ident)
                                nc.vector.tensor_copy(
                                    Ft[h][:, (i - TOPK) * 128:(i - TOPK + 1) * 128],
                                    ps_ft,
                                )

            # ---------------- phases 3+4 ------------------------------------
            _phase34(nc, tc, qT, kT, vN, trik4, ones_c, ones_cb, ones_r, Ft,
                     wo_sb, out_d)

    nc.compile()
    _CACHE[key] = nc
    return nc


def _phase34(nc, tc, qT, kT, vN, trik4, ones_c, ones_cb, ones_r, Ft, wo_sb, out_d):
    # ------- phase 3: attention with interleaved output projection -------
    # (wo(m) right after attn(m) so the output DMA spreads over the whole
    # kernel instead of piling into a DMA-bound tail phase)
    with (
        tc.tile_pool(name="att", bufs=8) as ap,
        tc.tile_pool(name="attb", bufs=2) as ab,
        tc.tile_pool(name="dtree", bufs=6) as dtp,
        tc.tile_pool(name="oTs", bufs=4) as otp,
        tc.tile_pool(name="outp", bufs=6) as outp,
        tc.tile_pool(name="att_s", bufs=2, space="PSUM") as pss,
        tc.tile_pool(name="att_o", bufs=3, space="PSUM") as pso,
        tc.tile_pool(name="att_d", bufs=1, space="PSUM") as psd,
    ):
        prev_wo = None  # deferred wo-section emitter (SW pipeline by 1 chunk)
        for m in range(NCHUNK):
            nsk = 8 if m < 2 else 4 * m + 4
            cols = slice(m * CH, (m + 1) * CH)
            ps_o = [pso.tile([128, CH], F32, tag="o", name=f"o{h}") for h in range(HPC)]
            oTc = [
                otp.tile([128, CH], F16, tag="oTc", name=f"oTc{h}")
                for h in range(HPC)
            ]
            # precompute the 4 diagonal-mask slabs for this chunk's band in
            # one broadcast matmul + one DVE max per head (off the exp->PV
            # critical path)
            mk4 = {}
            if m >= 2:
                for h in range(HPC):
                    ps_bc = pss.tile([128, 2, CH], F32, tag="s", name="bc")
                    nc.tensor.matmul(
                        ps_bc[:, 0, :],
                        ones_r,
                        Ft[h][:, (4 * m - TOPK) * 128:(4 * m - TOPK + 4) * 128],
                        start=True,
                        stop=True,
                    )
                    mk = ab.tile([128, CH], BF16, tag="mk4", bufs=2)
                    nc.vector.tensor_tensor(
                        mk, trik4, ps_bc[:, 0, :], op=mybir.AluOpType.max
                    )
                    mk4[h] = mk
            # one PSUM bank holds both heads' denominator rows (matmul output
            # base partition must be 0/32/64)
            ps_den2 = psd.tile([64, CH], F32, tag="den", name="den")
            ps_den = [ps_den2[32 * h:32 * h + 1, :] for h in range(HPC)]
            pexps = {}
            for jp in range(0, nsk, 2):
                for h in range(HPC):
                    ps_s2 = pss.tile([128, 2, CH], F32, tag="s", bufs=2)
                    for u in (0, 1):
                        j = jp + u
                        band = m >= 2 and j >= 4 * m
                        col0 = (j - 4 * m) * 128 if band else 0
                        nc.tensor.matmul(
                            ps_s2[:, u, col0:],
                            kT[:, j * 128:(j + 1) * 128],
                            qT[h][:, m * CH + col0:(m + 1) * CH],
                            start=True,
                            stop=True,
                        )
                    # one exp covers the j-pair; stale PSUM in the band's
                    # left margin exps to junk that no consumer reads
                    pexp2 = ap.tile([128, 2, CH], BF16, tag="pexp", bufs=8)
                    nc.scalar.activation(
                        out=pexp2,
                        in_=ps_s2,
                        func=mybir.ActivationFunctionType.Exp,
                        scale=INV_SQRT_D,
                    )
                    for u in (0, 1):
                        j = jp + u
                        band = m >= 2 and j >= 4 * m
                        col0 = (j - 4 * m) * 128 if band else 0
                        pexp = pexp2[:, u, :]
                        if band:
                            nc.vector.tensor_tensor(
                                pexp[:, col0:col0 + 128],
                                pexp[:, col0:col0 + 128],
                                mk4[h][:, col0:col0 + 128],
                                op=mybir.AluOpType.mult,
                            )
                            # band: per-j denominator on the valid columns
                            nc.tensor.matmul(
                                ps_den[h][:, col0:],
                                ones_cb,
                                pexp[:, col0:],
                                start=False,
                                stop=(j == nsk - 1),
                            )
                        else:
                            pexps[(j, h)] = pexp
                        nc.tensor.matmul(
                            ps_o[h][:, col0:],
                            vN[:, j, :],
                            pexp[:, col0:],
                            start=(j == 0),
                            stop=(j == nsk - 1),
                        )
                # dense quad complete: bf16 tree-sum on DVE, one den matmul
                j = jp + 1
                if (jp + 2) % 4 == 0 and not (m >= 2 and j >= 4 * m):
                    jq = jp - 2
                    for h in range(HPC):
                        t01 = dtp.tile([128, CH], BF16, tag="t01")
                        t23 = dtp.tile([128, CH], BF16, tag="t23")
                        nc.vector.tensor_tensor(
                            t01, pexps.pop((jq, h)), pexps.pop((jq + 1, h)),
                            op=mybir.AluOpType.add,
                        )
                        nc.vector.tensor_tensor(
                            t23, pexps.pop((jq + 2, h)), pexps.pop((jq + 3, h)),
                            op=mybir.AluOpType.add,
                        )
                        nc.vector.tensor_tensor(t01, t01, t23, op=mybir.AluOpType.add)
                        nc.tensor.matmul(
                            ps_den[h],
                            ones_cb,
                            t01,
                            start=(jq == 0),
                            stop=(m < 2 and j == nsk - 1),
                        )
            for h in range(HPC):
                rec = ab.tile([1, CH], F32R, tag="rec")
                nc.vector.reciprocal(rec, ps_den[h])
                ps_rb = pss.tile([128, 2, CH], F32, tag="s", name="rb")
                nc.tensor.matmul(ps_rb[:, 0, :], ones_r, rec, start=True, stop=True)
                bc_sb = ab.tile([128, CH], F32, tag="bcs")
                nc.scalar.copy(bc_sb, ps_rb[:, 0, :])
                nc.vector.tensor_tensor(
                    oTc[h], ps_o[h], bc_sb, op=mybir.AluOpType.mult
                )
            # ---- wo(m') emission: projection for s-tiles of chunk m' ----
            def emit_wo(mm, oTc_mm):
                for u in range(4):
                    st = 4 * mm + u
                    for n in range(4):
                        ncols = slice(n * 512, (n + 1) * 512)
                        ps_w = pso.tile([128, 512], F32, tag="o", name="psw")
                        nc.tensor.matmul(
                            ps_w,
                            oTc_mm[0][:, u * 128:(u + 1) * 128],
                            wo_sb[:, 0, ncols],
                            start=True,
                            stop=False,
                        )
                        nc.tensor.matmul(
                            ps_w,
                            oTc_mm[1][:, u * 128:(u + 1) * 128],
                            wo_sb[:, 1, ncols],
                            start=False,
                            stop=True,
                        )
                        osb = outp.tile([128, 512], F16, tag="ow")
                        if (st * 4 + n) % 2 == 0:
                            nc.scalar.copy(osb, ps_w)
                            nc.sync.dma_start(
                                out=out_d[st * 128:(st + 1) * 128, ncols], in_=osb
                            )
                        else:
                            nc.vector.tensor_copy(osb, ps_w)
                            nc.gpsimd.dma_start(
                                out=out_d[st * 128:(st + 1) * 128, ncols], in_=osb
                            )

            if prev_wo is not None:
                emit_wo(*prev_wo)
            prev_wo = (m, oTc)
        emit_wo(*prev_wo)
def kernel(x, wq, wk, wv, wo):
    bs = np.asarray(x).shape[0]
    in_maps = make_in_maps(x, wq, wk, wv, wo)
    nc = _build_nc()
    res = run_bass_kernel_spmd(nc, in_maps, list(range(N_CORES)))
    out = res.results[0]["out"].astype(np.float64)
    for c in range(1, N_CORES):
        out += res.results[c]["out"]
    return out.astype(np.float32).reshape(bs, SEQ, DIM)


if __name__ == "__main__":
    rng = np.random.default_rng(0)
    xs = {
        "x": rng.standard_normal((1, SEQ, DIM), dtype=np.float32),
        "wq": rng.standard_normal((DIM, DIM), dtype=np.float32) * (DIM ** -0.5),
        "wk": rng.standard_normal((DIM, DIM // 2), dtype=np.float32) * (DIM ** -0.5),
        "wv": rng.standard_normal((DIM, DIM // 2), dtype=np.float32) * (DIM ** -0.5),
        "wo": rng.standard_normal((DIM, DIM), dtype=np.float32) * (DIM ** -0.5),
    }
    out = kernel(**xs)
    print("out", out.shape, out.dtype, np.abs(out).max())



# revision 37
# speedup vs baseline: 3.1238x; 1.5215x over previous
"""MixtureOfBlockAttention TRN2 kernel — 8-core head-parallel (TP) Bass/Tile implementation.

Semantics (equivalent to the reference up to selection ties, measured rel
err ~1.96e-2 on HW, dominated by a handful of borderline top-k rows where
fp32r projection rounding flips the reference's fp32 gating decision):
the reference mask `maximum(token_mask, causal*NEG_INF)` masks a position iff
it is BOTH future AND in a non-selected block. Consequences:
  - query blocks 0..7 attend to ALL tokens of key blocks 0..7 (dense, no mask);
  - query block i>=8 attends densely to key blocks 0..i-1, and within its own
    (diagonal) block applies strict causal masking ONLY for rows whose own
    block is not among their top-8 gating blocks.
Selection rank for query s in block i (i>=8): own block selected iff
  #{j < i : g[s,j] > g[s,i]} < 8, with g = q . (block sums of roped k)
(positive-scale invariant, so block sums replace means and the 1/sqrt(d)
factor is dropped).

Sharding: 16 query heads / 8 cores = 2 heads per core; KV head c serves both.
wq/wk/wv column-sliced, wo row-sliced; partial outputs summed on host.
Host-side layout prep: x is transposed to xT[c, s] (the PE contracts over the
partition dim, so both matmul operands need c on partitions) and float inputs
are pre-rounded to fp32r; both are pure data-layout transforms.

Dtypes (chosen from a host-side error study; gating/score paths are kept
bit-identical to the fp32r baseline because the top-k margins are razor
thin, while smooth post-softmax paths run at 16 bits):
  - projections, rope, gating, QK scores: fp32r (1 PE cycle/row at N>=512)
  - exp output (pexp), V tiles: bf16 (exp can reach e^35, needs fp32 range)
  - softmax denominators: bf16 pairwise-tree quad sums on DVE + one
    [1,512] ones-matmul per quad (the old per-key-block ones-matmul burned
    ~63 us of PE per core)
  - exp is issued once per PAIR of key blocks from a 2-bank PSUM tile
    (halves ACT instruction overhead)
  - attention output + wo + output partials: fp16 (halves output DMA)
"""

import math
import sys

import numpy as np
from ml_dtypes import bfloat16 as ml_bf16

if "/opt/trn_rl_repo" not in sys.path:
    sys.path.insert(0, "/opt/trn_rl_repo")

import concourse.bacc as bacc
import concourse.mybir as mybir
import concourse.tile as tile
from concourse.bass_utils import run_bass_kernel_spmd
from concourse.masks import make_identity

F32 = mybir.dt.float32
F32R = mybir.dt.float32r
BF16 = mybir.dt.bfloat16
F16 = mybir.dt.float16

SEQ = 4096
DIM = 2048
HEAD_DIM = 128
N_HEADS = 16
N_CORES = 8
HPC = N_HEADS // N_CORES       # heads per core = 2
DPC = HPC * HEAD_DIM           # q/o dims per core = 256
BLOCK = 128
NB = SEQ // BLOCK              # 32 key blocks
TOPK = 8
NCHUNK = 8                     # s-chunks of 512
CH = SEQ // NCHUNK             # 512
NCT = DIM // 128               # 16 contraction tiles
INV_SQRT_D = 1.0 / math.sqrt(HEAD_DIM)

_CACHE = {}


def _round_fp32r(a):
    """Round fp32 to the fp32r grid (top-11-bit mantissa, round-to-nearest)."""
    a = np.ascontiguousarray(a, dtype=np.float32)
    try:
        from neuron_dtypes import static_cast_fp32_to_fp32r

        return static_cast_fp32_to_fp32r(a).view(np.float32).astype(np.float32)
    except Exception:
        u = a.view(np.uint32)
        return ((u + np.uint32(0x800)) & np.uint32(0xFFFFF000)).view(np.float32).copy()


def _host_constants():
    if "consts" in _CACHE:
        return _CACHE["consts"]
    p = np.arange(HEAD_DIM // 2, dtype=np.float64)
    inv_freq = 1.0 / (10000.0 ** (2.0 * p / HEAD_DIM))
    ang = np.arange(SEQ, dtype=np.float64)[None, :] * inv_freq[:, None]  # [64, S]
    cos = np.cos(ang).astype(np.float32)
    sin = np.sin(ang).astype(np.float32)
    cos_ds = np.ascontiguousarray(np.repeat(cos, 2, axis=0))   # [128, S]
    sin_ds = np.empty((HEAD_DIM, SEQ), dtype=np.float32)       # signed sin
    sin_ds[0::2] = -sin
    sin_ds[1::2] = sin
    pswap = np.zeros((128, 128), dtype=np.float32)             # swap 2p <-> 2p+1
    idx = np.arange(128)
    pswap[idx, idx ^ 1] = 1.0
    r = np.arange(BLOCK)
    trikeep = (r[:, None] <= r[None, :]).astype(np.float32)    # keep iff sk <= sq
    trik4 = np.tile(trikeep, (1, 4))                           # [128, 512]
    ones_col = np.ones((128, 1), dtype=np.float32)
    ones_row = np.ones((1, 128), dtype=np.float32)
    _CACHE["consts"] = (cos_ds, sin_ds, pswap, trik4, ones_col, ones_row)
    return _CACHE["consts"]


def make_in_maps(x, wq, wk, wv, wo):
    """Shard + lay out the full inputs for the 8 cores."""
    x2 = np.asarray(x, dtype=np.float32).reshape(SEQ, DIM)
    xT = _round_fp32r(np.ascontiguousarray(x2.T))
    wq = np.asarray(wq, dtype=np.float32)
    wk = np.asarray(wk, dtype=np.float32)
    wv = np.asarray(wv, dtype=np.float32)
    wo = np.asarray(wo, dtype=np.float32)
    cos_ds, sin_ds, pswap, trik4, ones_col, ones_row = _host_constants()
    pswap_r = _round_fp32r(pswap)
    ones_col_r = _round_fp32r(ones_col)
    ones_row_r = _round_fp32r(ones_row)
    in_maps = []
    for c in range(N_CORES):
        in_maps.append(
            {
                "xT": xT,
                "wq": _round_fp32r(wq[:, c * DPC:(c + 1) * DPC]),
                "wk": _round_fp32r(wk[:, c * HEAD_DIM:(c + 1) * HEAD_DIM]),
                "wv": _round_fp32r(wv[:, c * HEAD_DIM:(c + 1) * HEAD_DIM]),
                "wo": np.asarray(wo[c * DPC:(c + 1) * DPC, :], dtype=np.float16),
                "cos_ds": cos_ds,
                "sin_ds": sin_ds,
                "pswap": pswap_r,
                "trik4": trik4,
                "ones_c": ones_col_r,
                "ones_cb": ones_col.astype(ml_bf16),
                "ones_r": ones_row_r,
            }
        )
    return in_maps


def _build_nc(reps=1):
    key = f"nc{reps}"
    if key in _CACHE:
        return _CACHE[key]
    nc = bacc.Bacc(None, target_bir_lowering=False)

    xT_d = nc.dram_tensor("xT", [DIM, SEQ], F32R, kind="ExternalInput")
    wq_d = nc.dram_tensor("wq", [DIM, DPC], F32R, kind="ExternalInput")
    wk_d = nc.dram_tensor("wk", [DIM, HEAD_DIM], F32R, kind="ExternalInput")
    wv_d = nc.dram_tensor("wv", [DIM, HEAD_DIM], F32R, kind="ExternalInput")
    wo_d = nc.dram_tensor("wo", [DPC, DIM], F16, kind="ExternalInput")
    cos_d = nc.dram_tensor("cos_ds", [HEAD_DIM, SEQ], F32, kind="ExternalInput")
    sin_d = nc.dram_tensor("sin_ds", [HEAD_DIM, SEQ], F32, kind="ExternalInput")
    psw_d = nc.dram_tensor("pswap", [128, 128], F32R, kind="ExternalInput")
    trk_d = nc.dram_tensor("trik4", [BLOCK, 4 * BLOCK], F32, kind="ExternalInput")
    onc_d = nc.dram_tensor("ones_c", [128, 1], F32R, kind="ExternalInput")
    oncb_d = nc.dram_tensor("ones_cb", [128, 1], BF16, kind="ExternalInput")
    onr_d = nc.dram_tensor("ones_r", [1, 128], F32R, kind="ExternalInput")
    out_d = nc.dram_tensor("out", [SEQ, DIM], F16, kind="ExternalOutput")

    with tile.TileContext(nc) as tc, nc.allow_low_precision(
        reason="float32r rounding of matmul operands is intentional"
    ):
      for _rep in range(reps):
        with tc.tile_pool(name="persist", bufs=1) as per:
            qT = [per.tile([128, SEQ], F32R, tag=f"qT{h}", name=f"qT{h}") for h in range(HPC)]
            kT = per.tile([128, SEQ], F32R, tag="kT")
            vN = per.tile([128, NB, 128], BF16, tag="vN")   # [s-in-tile, sk-tile, d]
            ident = per.tile([128, 128], F32, tag="ident")
            pswap = per.tile([128, 128], F32R, tag="pswap")
            trik4 = per.tile([BLOCK, 4 * BLOCK], F32, tag="trik4")
            ones_c = per.tile([128, 1], F32R, tag="ones_c")
            ones_cb = per.tile([128, 1], BF16, tag="ones_cb")
            ones_r = per.tile([1, 128], F32R, tag="ones_r")
            bm = per.tile([128, NB], F32R, tag="bm")
            # per-head notflag rows: Ft[h][0, (i-TOPK)*128:...] is the [1,128]
            # notflag row for query block i, at base partition 0
            Ft = [
                per.tile([1, (NB - TOPK) * 128], F32R, tag=f"Ft{h}", name=f"Ft{h}")
                for h in range(HPC)
            ]

            # wo is needed only in phase 3 but its DMA is issued up front so
            # the transfer hides under phase 1
            wo_sb = per.tile([128, HPC, DIM], F16, tag="wo_sb")
            nc.scalar.dma_start(
                out=wo_sb, in_=wo_d.rearrange("(t p) d -> p t d", p=128)
            )
            make_identity(nc, ident)
            nc.gpsimd.dma_start(out=pswap, in_=psw_d[:])
            nc.gpsimd.dma_start(out=trik4, in_=trk_d[:])
            nc.gpsimd.dma_start(out=ones_c, in_=onc_d[:])
            nc.gpsimd.dma_start(out=ones_cb, in_=oncb_d[:])
            nc.gpsimd.dma_start(out=ones_r, in_=onr_d[:])

            # ---------------- phase 1: projections + rope -------------------
            with (
                tc.tile_pool(name="wpool", bufs=1) as wp,
                tc.tile_pool(name="xtp", bufs=10) as xtp,
                tc.tile_pool(name="ropep", bufs=2) as rp,
                tc.tile_pool(name="csin", bufs=2) as csp,
                tc.tile_pool(name="pj_ps", bufs=2, space="PSUM") as trps,
                tc.tile_pool(name="acc_ps", bufs=4, space="PSUM") as accps,
            ):
                wq_sb = wp.tile([128, NCT, DPC], F32R, tag="wq")
                wk_sb = wp.tile([128, NCT, HEAD_DIM], F32R, tag="wk")
                wv_sb = wp.tile([128, NCT, HEAD_DIM], F32R, tag="wv")
                wq_r = wq_d.rearrange("(t p) d -> p t d", p=128)
                nc.gpsimd.dma_start(out=wq_sb[:, 0:4, :], in_=wq_r[:, 0:4, :])
                nc.gpsimd.dma_start(out=wq_sb[:, 4:16, :], in_=wq_r[:, 4:16, :])
                nc.gpsimd.dma_start(
                    out=wk_sb, in_=wk_d.rearrange("(t p) d -> p t d", p=128)
                )
                nc.gpsimd.dma_start(
                    out=wv_sb, in_=wv_d.rearrange("(t p) d -> p t d", p=128)
                )

                gp = wp  # reuse the bufs=1 pool scope for small gating tiles
                for m in range(NCHUNK):
                    cols = slice(m * CH, (m + 1) * CH)
                    ps_q0 = accps.tile([128, CH], F32, tag="acc")
                    ps_q1 = accps.tile([128, CH], F32, tag="acc")
                    ps_k = accps.tile([128, CH], F32, tag="acc")
                    ps_v = accps.tile([128, CH], F32, tag="acc")
                    for cc in range(NCT):
                        xt = xtp.tile([128, CH], F32R, tag="xt")
                        nc.sync.dma_start(
                            out=xt, in_=xT_d[cc * 128:(cc + 1) * 128, cols]
                        )
                        st0, sp0 = (cc == 0), (cc == NCT - 1)
                        nc.tensor.matmul(ps_q0, wq_sb[:, cc, 0:128], xt, start=st0, stop=sp0)
                        nc.tensor.matmul(ps_q1, wq_sb[:, cc, 128:256], xt, start=st0, stop=sp0)
                        nc.tensor.matmul(ps_k, wk_sb[:, cc, :], xt, start=st0, stop=sp0)
                        nc.tensor.matmul(ps_v, wv_sb[:, cc, :], xt, start=st0, stop=sp0)

                    cos_t = csp.tile([128, CH], F32, tag="cos")
                    nc.gpsimd.dma_start(out=cos_t, in_=cos_d[:, cols])
                    sin_t = csp.tile([128, CH], F32, tag="sin")
                    nc.gpsimd.dma_start(out=sin_t, in_=sin_d[:, cols])

                    for psrc, dstT in ((ps_q0, qT[0]), (ps_q1, qT[1]), (ps_k, kT)):
                        raw = rp.tile([128, CH], F32R, tag="qraw")
                        nc.vector.tensor_copy(raw, psrc)
                        ps_sw = trps.tile([128, CH], F32, tag="tr")
                        nc.tensor.matmul(ps_sw, pswap, raw, start=True, stop=True)
                        t2 = rp.tile([128, CH], F32, tag="t2")
                        nc.vector.tensor_tensor(
                            t2, raw.bitcast(F32), cos_t, op=mybir.AluOpType.mult
                        )
                        # sw *= sin in place (PSUM), then add -> rope output
                        nc.vector.tensor_tensor(ps_sw, ps_sw, sin_t, op=mybir.AluOpType.mult)
                        nc.vector.tensor_tensor(
                            dstT[:, cols], t2, ps_sw, op=mybir.AluOpType.add
                        )

                    # V: evacuate then PE-transpose to natural [s, d] layout
                    vtmp = rp.tile([128, CH], F32, tag="qraw2")
                    nc.vector.tensor_copy(vtmp, ps_v)
                    ps_vt = trps.tile([128, CH], F32, tag="tr")
                    for u in range(4):
                        nc.tensor.transpose(
                            ps_vt[:, u * 128:(u + 1) * 128],
                            vtmp[:, u * 128:(u + 1) * 128],
                            ident,
                        )
                    nc.vector.tensor_copy(
                        vN[:, 4 * m:4 * m + 4, :],
                        ps_vt.rearrange("p (u d) -> p u d", u=4),
                    )

                    # partial block sums for this chunk's 4 key blocks
                    nc.vector.tensor_reduce(
                        bm[:, 4 * m:4 * m + 4],
                        kT.bitcast(F32)[:, cols].rearrange("p (b t) -> p b t", b=4),
                        axis=mybir.AxisListType.X,
                        op=mybir.AluOpType.add,
                    )
                    # gating flags for this chunk's query blocks (needs bm 0..i)
                    if m >= 2:
                        for h in range(HPC):
                            for i in range(4 * m, 4 * m + 4):
                                nbk = 4 * m + 4  # even N; cols > i unused
                                ps_g = trps.tile([128, NB], F32, tag="g", bufs=1)
                                nc.tensor.matmul(
                                    ps_g[:, 0:nbk],
                                    qT[h][:, i * 128:(i + 1) * 128],
                                    bm[:, 0:nbk],
                                    start=True,
                                    stop=True,
                                )
                                cmp = gp.tile([128, NB], F32, tag="cmp", bufs=2)
                                cnt = gp.tile([128, 1], F32, tag="cnt", bufs=2)
                                nc.vector.tensor_scalar(
                                    out=cmp[:, 0:i],
                                    in0=ps_g[:, 0:i],
                                    scalar1=ps_g[:, i:i + 1],
                                    scalar2=None,
                                    op0=mybir.AluOpType.is_gt,
                                )
                                nc.vector.tensor_reduce(
                                    cnt,
                                    cmp[:, 0:i],
                                    axis=mybir.AxisListType.X,
                                    op=mybir.AluOpType.add,
                                )
                                # notflag: 1.0 -> own block selected (keep all)
                                nf = gp.tile([128, 1], F32, tag="nf", bufs=2)
                                nc.vector.tensor_scalar(
                                    out=nf,
                                    in0=cnt,
                                    scalar1=float(TOPK) - 0.5,
                                    scalar2=None,
                                    op0=mybir.AluOpType.is_lt,
                                )
                                ps_ft = trps.tile([1, 128], F32, tag="ft", bufs=1)
                                nc.tensor.transpose(ps_ft, nf, ident)
                                nc.vector.tensor_copy(
                                    Ft[h][:, (i - TOPK) * 128:(i - TOPK + 1) * 128],
                                    ps_ft,
                                )

            # ---------------- phases 3+4 ------------------------------------
            _phase34(nc, tc, qT, kT, vN, trik4, ones_c, ones_cb, ones_r, Ft,
                     wo_sb, out_d)

    nc.compile()
    _CACHE[key] = nc
    return nc


def _phase34(nc, tc, qT, kT, vN, trik4, ones_c, ones_cb, ones_r, Ft, wo_sb, out_d):
    # ------- phase 3: attention with interleaved output projection -------
    # (wo(m) right after attn(m) so the output DMA spreads over the whole
    # kernel instead of piling into a DMA-bound tail phase)
    with (
        tc.tile_pool(name="att", bufs=8) as ap,
        tc.tile_pool(name="attb", bufs=2) as ab,
        tc.tile_pool(name="dtree", bufs=6) as dtp,
        tc.tile_pool(name="oTs", bufs=4) as otp,
        tc.tile_pool(name="outp", bufs=6) as outp,
        tc.tile_pool(name="att_s", bufs=2, space="PSUM") as pss,
        tc.tile_pool(name="att_o", bufs=3, space="PSUM") as pso,
        tc.tile_pool(name="att_d", bufs=1, space="PSUM") as psd,
    ):
        # wo(m-1) work items are interleaved between chunk m's attention
        # pairs: the PE executes in program order, so without this the PE
        # sits idle ~300 ns per pair waiting on the ACT exp, and the wo
        # matmuls pile into a serial section at each chunk boundary.
        def emit_one_wo(oTc_mm, st, n):
            ncols = slice(n * 512, (n + 1) * 512)
            ps_w = pso.tile([128, 512], F32, tag="o", name="psw")
            act_evac = (st * 4 + n) % 2 == 0
            nc.tensor.matmul(
                ps_w,
                oTc_mm[0][:, (st % 4) * 128:(st % 4 + 1) * 128],
                wo_sb[:, 0, ncols],
                start=True,
                stop=False,
            )
            nc.tensor.matmul(
                ps_w,
                oTc_mm[1][:, (st % 4) * 128:(st % 4 + 1) * 128],
                wo_sb[:, 1, ncols],
                start=False,
                stop=True,
            )
            osb = outp.tile([128, 512], F16, tag="ow")
            # mid-chunk items must NOT evacuate on ACT: the in-order ACT
            # stream is the attention pacer (exp), a copy there stalls it
            if act_evac:
                nc.scalar.copy(osb, ps_w)
                nc.sync.dma_start(
                    out=out_d[st * 128:(st + 1) * 128, ncols], in_=osb
                )
            else:
                nc.vector.tensor_copy(osb, ps_w)
                nc.gpsimd.dma_start(
                    out=out_d[st * 128:(st + 1) * 128, ncols], in_=osb
                )

        prev_wo = None  # (m-1, its oTc tiles); items drain during chunk m
        for m in range(NCHUNK):
            nsk = 8 if m < 2 else 4 * m + 4
            cols = slice(m * CH, (m + 1) * CH)
            ps_o = [pso.tile([128, CH], F32, tag="o", name=f"o{h}") for h in range(HPC)]
            oTc = [
                otp.tile([128, CH], F16, tag="oTc", name=f"oTc{h}")
                for h in range(HPC)
            ]
            # precompute the 4 diagonal-mask slabs for this chunk's band in
            # one broadcast matmul + one DVE max per head (off the exp->PV
            # critical path)
            mk4 = {}
            if m >= 2:
                for h in range(HPC):
                    ps_bc = pss.tile([128, 2, CH], F32, tag="s", name="bc")
                    nc.tensor.matmul(
                        ps_bc[:, 0, :],
                        ones_r,
                        Ft[h][:, (4 * m - TOPK) * 128:(4 * m - TOPK + 4) * 128],
                        start=True,
                        stop=True,
                    )
                    mk = ab.tile([128, CH], BF16, tag="mk4", bufs=2)
                    nc.vector.tensor_tensor(
                        mk, trik4, ps_bc[:, 0, :], op=mybir.AluOpType.max
                    )
                    mk4[h] = mk
            # one PSUM bank holds both heads' denominator rows (matmul output
            # base partition must be 0/32/64)
            ps_den2 = psd.tile([64, CH], F32, tag="den", name="den")
            ps_den = [ps_den2[32 * h:32 * h + 1, :] for h in range(HPC)]
            pexps = {}
            for jp in range(0, nsk, 2):
                for h in range(HPC):
                    ps_s2 = pss.tile([128, 2, CH], F32, tag="s", bufs=2)
                    for u in (0, 1):
                        j = jp + u
                        band = m >= 2 and j >= 4 * m
                        col0 = (j - 4 * m) * 128 if band else 0
                        nc.tensor.matmul(
                            ps_s2[:, u, col0:],
                            kT[:, j * 128:(j + 1) * 128],
                            qT[h][:, m * CH + col0:(m + 1) * CH],
                            start=True,
                            stop=True,
                        )
                    # one exp covers the j-pair; stale PSUM in the band's
                    # left margin exps to junk that no consumer reads
                    pexp2 = ap.tile([128, 2, CH], BF16, tag="pexp", bufs=8)
                    nc.scalar.activation(
                        out=pexp2,
                        in_=ps_s2,
                        func=mybir.ActivationFunctionType.Exp,
                        scale=INV_SQRT_D,
                    )
                    for u in (0, 1):
                        j = jp + u
                        band = m >= 2 and j >= 4 * m
                        col0 = (j - 4 * m) * 128 if band else 0
                        pexp = pexp2[:, u, :]
                        if band:
                            nc.vector.tensor_tensor(
                                pexp[:, col0:col0 + 128],
                                pexp[:, col0:col0 + 128],
                                mk4[h][:, col0:col0 + 128],
                                op=mybir.AluOpType.mult,
                            )
                            # band: per-j denominator on the valid columns
                            nc.tensor.matmul(
                                ps_den[h][:, col0:],
                                ones_cb,
                                pexp[:, col0:],
                                start=False,
                                stop=(j == nsk - 1),
                            )
                        else:
                            pexps[(j, h)] = pexp
                        nc.tensor.matmul(
                            ps_o[h][:, col0:],
                            vN[:, j, :],
                            pexp[:, col0:],
                            start=(j == 0),
                            stop=(j == nsk - 1),
                        )
                # dense quad complete: bf16 tree-sum on DVE, one den matmul
                j = jp + 1
                if (jp + 2) % 4 == 0 and not (m >= 2 and j >= 4 * m):
                    jq = jp - 2
                    for h in range(HPC):
                        t01 = dtp.tile([128, CH], BF16, tag="t01")
                        t23 = dtp.tile([128, CH], BF16, tag="t23")
                        nc.vector.tensor_tensor(
                            t01, pexps.pop((jq, h)), pexps.pop((jq + 1, h)),
                            op=mybir.AluOpType.add,
                        )
                        nc.vector.tensor_tensor(
                            t23, pexps.pop((jq + 2, h)), pexps.pop((jq + 3, h)),
                            op=mybir.AluOpType.add,
                        )
                        nc.vector.tensor_tensor(t01, t01, t23, op=mybir.AluOpType.add)
                        nc.tensor.matmul(
                            ps_den[h],
                            ones_cb,
                            t01,
                            start=(jq == 0),
                            stop=(m < 2 and j == nsk - 1),
                        )
            for h in range(HPC):
                rec = ab.tile([1, CH], F32R, tag="rec")
                nc.vector.reciprocal(rec, ps_den[h])
                ps_rb = pss.tile([128, 2, CH], F32, tag="s", name="rb")
                nc.tensor.matmul(ps_rb[:, 0, :], ones_r, rec, start=True, stop=True)
                bc_sb = ab.tile([128, CH], F32, tag="bcs")
                nc.scalar.copy(bc_sb, ps_rb[:, 0, :])
                nc.vector.tensor_tensor(
                    oTc[h], ps_o[h], bc_sb, op=mybir.AluOpType.mult
                )
            if prev_wo is not None:
                for u in range(4):
                    for n in range(4):
                        emit_one_wo(prev_wo[1], 4 * prev_wo[0] + u, n)
            prev_wo = (m, oTc)
        for u in range(4):
            for n in range(4):
                emit_one_wo(prev_wo[1], 4 * prev_wo[0] + u, n)
def kernel(x, wq, wk, wv, wo):
    bs = np.asarray(x).shape[0]
    in_maps = make_in_maps(x, wq, wk, wv, wo)
    nc = _build_nc()
    res = run_bass_kernel_spmd(nc, in_maps, list(range(N_CORES)))
    out = res.results[0]["out"].astype(np.float64)
    for c in range(1, N_CORES):
        out += res.results[c]["out"]
    return out.astype(np.float32).reshape(bs, SEQ, DIM)


if __name__ == "__main__":
    rng = np.random.default_rng(0)
    xs = {
        "x": rng.standard_normal((1, SEQ, DIM), dtype=np.float32),
        "wq": rng.standard_normal((DIM, DIM), dtype=np.float32) * (DIM ** -0.5),
        "wk": rng.standard_normal((DIM, DIM // 2), dtype=np.float32) * (DIM ** -0.5),
        "wv": rng.standard_normal((DIM, DIM // 2), dtype=np.float32) * (DIM ** -0.5),
        "wo": rng.standard_normal((DIM, DIM), dtype=np.float32) * (DIM ** -0.5),
    }
    out = kernel(**xs)
    print("out", out.shape, out.dtype, np.abs(out).max())



# revision 41
# speedup vs baseline: 3.2578x; 1.0429x over previous
"""MixtureOfBlockAttention TRN2 kernel — 8-core head-parallel (TP) Bass/Tile implementation.

Semantics (equivalent to the reference up to selection ties, measured rel
err ~1.96e-2 on HW, dominated by a handful of borderline top-k rows where
fp32r projection rounding flips the reference's fp32 gating decision):
the reference mask `maximum(token_mask, causal*NEG_INF)` masks a position iff
it is BOTH future AND in a non-selected block. Consequences:
  - query blocks 0..7 attend to ALL tokens of key blocks 0..7 (dense, no mask);
  - query block i>=8 attends densely to key blocks 0..i-1, and within its own
    (diagonal) block applies strict causal masking ONLY for rows whose own
    block is not among their top-8 gating blocks.
Selection rank for query s in block i (i>=8): own block selected iff
  #{j < i : g[s,j] > g[s,i]} < 8, with g = q . (block sums of roped k)
(positive-scale invariant, so block sums replace means and the 1/sqrt(d)
factor is dropped).

Sharding: 16 query heads / 8 cores = 2 heads per core; KV head c serves both.
wq/wk/wv column-sliced, wo row-sliced; partial outputs summed on host.
Host-side layout prep: x is transposed to xT[c, s] (the PE contracts over the
partition dim, so both matmul operands need c on partitions) and float inputs
are pre-rounded to fp32r; both are pure data-layout transforms.

Dtypes (chosen from a host-side error study; gating/score paths are kept
bit-identical to the fp32r baseline because the top-k margins are razor
thin, while smooth post-softmax paths run at 16 bits):
  - projections, rope, gating, QK scores: fp32r (1 PE cycle/row at N>=512)
  - exp output (pexp), V tiles: bf16 (exp can reach e^35, needs fp32 range)
  - softmax denominators: bf16 pairwise-tree quad sums on DVE + one
    [1,512] ones-matmul per quad (the old per-key-block ones-matmul burned
    ~63 us of PE per core)
  - exp is issued once per PAIR of key blocks from a 2-bank PSUM tile
    (halves ACT instruction overhead)
  - attention output + wo + output partials: fp16 (halves output DMA)
"""

import math
import sys

import numpy as np
from ml_dtypes import bfloat16 as ml_bf16

if "/opt/trn_rl_repo" not in sys.path:
    sys.path.insert(0, "/opt/trn_rl_repo")

import concourse.bacc as bacc
import concourse.mybir as mybir
import concourse.tile as tile
from concourse.bass_utils import run_bass_kernel_spmd
from concourse.masks import make_identity

F32 = mybir.dt.float32
F32R = mybir.dt.float32r
BF16 = mybir.dt.bfloat16
F16 = mybir.dt.float16

SEQ = 4096
DIM = 2048
HEAD_DIM = 128
N_HEADS = 16
N_CORES = 8
HPC = N_HEADS // N_CORES       # heads per core = 2
DPC = HPC * HEAD_DIM           # q/o dims per core = 256
BLOCK = 128
NB = SEQ // BLOCK              # 32 key blocks
TOPK = 8
NCHUNK = 8                     # s-chunks of 512
CH = SEQ // NCHUNK             # 512
NCT = DIM // 128               # 16 contraction tiles
INV_SQRT_D = 1.0 / math.sqrt(HEAD_DIM)

_CACHE = {}


def _round_fp32r(a):
    """Round fp32 to the fp32r grid (top-11-bit mantissa, round-to-nearest)."""
    a = np.ascontiguousarray(a, dtype=np.float32)
    try:
        from neuron_dtypes import static_cast_fp32_to_fp32r

        return static_cast_fp32_to_fp32r(a).view(np.float32).astype(np.float32)
    except Exception:
        u = a.view(np.uint32)
        return ((u + np.uint32(0x800)) & np.uint32(0xFFFFF000)).view(np.float32).copy()


def _host_constants():
    if "consts" in _CACHE:
        return _CACHE["consts"]
    p = np.arange(HEAD_DIM // 2, dtype=np.float64)
    inv_freq = 1.0 / (10000.0 ** (2.0 * p / HEAD_DIM))
    ang = np.arange(SEQ, dtype=np.float64)[None, :] * inv_freq[:, None]  # [64, S]
    cos = np.cos(ang).astype(np.float32)
    sin = np.sin(ang).astype(np.float32)
    cos_ds = np.ascontiguousarray(np.repeat(cos, 2, axis=0))   # [128, S]
    sin_ds = np.empty((HEAD_DIM, SEQ), dtype=np.float32)       # signed sin
    sin_ds[0::2] = -sin
    sin_ds[1::2] = sin
    pswap = np.zeros((128, 128), dtype=np.float32)             # swap 2p <-> 2p+1
    idx = np.arange(128)
    pswap[idx, idx ^ 1] = 1.0
    r = np.arange(BLOCK)
    trikeep = (r[:, None] <= r[None, :]).astype(np.float32)    # keep iff sk <= sq
    trik4 = np.tile(trikeep, (1, 4))                           # [128, 512]
    ones_col = np.ones((128, 1), dtype=np.float32)
    ones_row = np.ones((1, 128), dtype=np.float32)
    _CACHE["consts"] = (cos_ds, sin_ds, pswap, trik4, ones_col, ones_row)
    return _CACHE["consts"]


def make_in_maps(x, wq, wk, wv, wo):
    """Shard + lay out the full inputs for the 8 cores."""
    x2 = np.asarray(x, dtype=np.float32).reshape(SEQ, DIM)
    xT = _round_fp32r(np.ascontiguousarray(x2.T))
    wq = np.asarray(wq, dtype=np.float32)
    wk = np.asarray(wk, dtype=np.float32)
    wv = np.asarray(wv, dtype=np.float32)
    wo = np.asarray(wo, dtype=np.float32)
    cos_ds, sin_ds, pswap, trik4, ones_col, ones_row = _host_constants()
    pswap_r = _round_fp32r(pswap)
    ones_col_r = _round_fp32r(ones_col)
    ones_row_r = _round_fp32r(ones_row)
    in_maps = []
    for c in range(N_CORES):
        in_maps.append(
            {
                "xT": xT,
                "wq": _round_fp32r(wq[:, c * DPC:(c + 1) * DPC]),
                "wk": _round_fp32r(wk[:, c * HEAD_DIM:(c + 1) * HEAD_DIM]),
                "wv": _round_fp32r(wv[:, c * HEAD_DIM:(c + 1) * HEAD_DIM]),
                "wo": np.asarray(wo[c * DPC:(c + 1) * DPC, :], dtype=np.float16),
                "cos_ds": cos_ds,
                "sin_ds": sin_ds,
                "pswap": pswap_r,
                "trik4": trik4,
                "ones_c": ones_col_r,
                "ones_cb": ones_col.astype(ml_bf16),
                "ones_r": ones_row_r,
            }
        )
    return in_maps


def _build_nc(reps=1):
    key = f"nc{reps}"
    if key in _CACHE:
        return _CACHE[key]
    nc = bacc.Bacc(None, target_bir_lowering=False)

    xT_d = nc.dram_tensor("xT", [DIM, SEQ], F32R, kind="ExternalInput")
    wq_d = nc.dram_tensor("wq", [DIM, DPC], F32R, kind="ExternalInput")
    wk_d = nc.dram_tensor("wk", [DIM, HEAD_DIM], F32R, kind="ExternalInput")
    wv_d = nc.dram_tensor("wv", [DIM, HEAD_DIM], F32R, kind="ExternalInput")
    wo_d = nc.dram_tensor("wo", [DPC, DIM], F16, kind="ExternalInput")
    cos_d = nc.dram_tensor("cos_ds", [HEAD_DIM, SEQ], F32, kind="ExternalInput")
    sin_d = nc.dram_tensor("sin_ds", [HEAD_DIM, SEQ], F32, kind="ExternalInput")
    psw_d = nc.dram_tensor("pswap", [128, 128], F32R, kind="ExternalInput")
    trk_d = nc.dram_tensor("trik4", [BLOCK, 4 * BLOCK], F32, kind="ExternalInput")
    onc_d = nc.dram_tensor("ones_c", [128, 1], F32R, kind="ExternalInput")
    oncb_d = nc.dram_tensor("ones_cb", [128, 1], BF16, kind="ExternalInput")
    onr_d = nc.dram_tensor("ones_r", [1, 128], F32R, kind="ExternalInput")
    out_d = nc.dram_tensor("out", [SEQ, DIM], F16, kind="ExternalOutput")

    with tile.TileContext(nc) as tc, nc.allow_low_precision(
        reason="float32r rounding of matmul operands is intentional"
    ):
      for _rep in range(reps):
        with tc.tile_pool(name="persist", bufs=1) as per:
            qT = [per.tile([128, SEQ], F32R, tag=f"qT{h}", name=f"qT{h}") for h in range(HPC)]
            kT = per.tile([128, SEQ], F32R, tag="kT")
            vN = per.tile([128, NB, 128], BF16, tag="vN")   # [s-in-tile, sk-tile, d]
            ident = per.tile([128, 128], F32, tag="ident")
            pswap = per.tile([128, 128], F32R, tag="pswap")
            trik4 = per.tile([BLOCK, 4 * BLOCK], F32, tag="trik4")
            ones_c = per.tile([128, 1], F32R, tag="ones_c")
            ones_cb = per.tile([128, 1], BF16, tag="ones_cb")
            ones_r = per.tile([1, 128], F32R, tag="ones_r")
            bm = per.tile([128, NB], F32R, tag="bm")
            # per-head notflag rows: Ft[h][0, (i-TOPK)*128:...] is the [1,128]
            # notflag row for query block i, at base partition 0
            Ft = [
                per.tile([1, (NB - TOPK) * 128], F32R, tag=f"Ft{h}", name=f"Ft{h}")
                for h in range(HPC)
            ]

            # wo is needed only in phase 3 but its DMA is issued up front so
            # the transfer hides under phase 1
            wo_sb = per.tile([128, HPC, DIM], F16, tag="wo_sb")
            nc.scalar.dma_start(
                out=wo_sb, in_=wo_d.rearrange("(t p) d -> p t d", p=128)
            )
            make_identity(nc, ident)
            nc.gpsimd.dma_start(out=pswap, in_=psw_d[:])
            nc.gpsimd.dma_start(out=trik4, in_=trk_d[:])
            nc.gpsimd.dma_start(out=ones_c, in_=onc_d[:])
            nc.gpsimd.dma_start(out=ones_cb, in_=oncb_d[:])
            nc.gpsimd.dma_start(out=ones_r, in_=onr_d[:])

            # ---------------- phase 1: projections + rope -------------------
            with (
                tc.tile_pool(name="wpool", bufs=1) as wp,
                tc.tile_pool(name="xtp", bufs=10) as xtp,
                tc.tile_pool(name="ropep", bufs=2) as rp,
                tc.tile_pool(name="csin", bufs=2) as csp,
                tc.tile_pool(name="pj_ps", bufs=2, space="PSUM") as trps,
                tc.tile_pool(name="acc_ps", bufs=4, space="PSUM") as accps,
            ):
                wq_sb = wp.tile([128, NCT, DPC], F32R, tag="wq")
                wk_sb = wp.tile([128, NCT, HEAD_DIM], F32R, tag="wk")
                wv_sb = wp.tile([128, NCT, HEAD_DIM], F32R, tag="wv")
                wq_r = wq_d.rearrange("(t p) d -> p t d", p=128)
                nc.gpsimd.dma_start(out=wq_sb[:, 0:4, :], in_=wq_r[:, 0:4, :])
                nc.gpsimd.dma_start(out=wq_sb[:, 4:16, :], in_=wq_r[:, 4:16, :])
                nc.gpsimd.dma_start(
                    out=wk_sb, in_=wk_d.rearrange("(t p) d -> p t d", p=128)
                )
                nc.gpsimd.dma_start(
                    out=wv_sb, in_=wv_d.rearrange("(t p) d -> p t d", p=128)
                )

                gp = wp  # reuse the bufs=1 pool scope for small gating tiles
                for m in range(NCHUNK):
                    cols = slice(m * CH, (m + 1) * CH)
                    ps_q0 = accps.tile([128, CH], F32, tag="acc")
                    ps_q1 = accps.tile([128, CH], F32, tag="acc")
                    ps_k = accps.tile([128, CH], F32, tag="acc")
                    ps_v = accps.tile([128, CH], F32, tag="acc")
                    for cc in range(NCT):
                        xt = xtp.tile([128, CH], F32R, tag="xt")
                        nc.sync.dma_start(
                            out=xt, in_=xT_d[cc * 128:(cc + 1) * 128, cols]
                        )
                        st0, sp0 = (cc == 0), (cc == NCT - 1)
                        nc.tensor.matmul(ps_q0, wq_sb[:, cc, 0:128], xt, start=st0, stop=sp0)
                        nc.tensor.matmul(ps_q1, wq_sb[:, cc, 128:256], xt, start=st0, stop=sp0)
                        nc.tensor.matmul(ps_k, wk_sb[:, cc, :], xt, start=st0, stop=sp0)
                        nc.tensor.matmul(ps_v, wv_sb[:, cc, :], xt, start=st0, stop=sp0)

                    cos_t = csp.tile([128, CH], F32, tag="cos")
                    nc.gpsimd.dma_start(out=cos_t, in_=cos_d[:, cols])
                    sin_t = csp.tile([128, CH], F32, tag="sin")
                    nc.gpsimd.dma_start(out=sin_t, in_=sin_d[:, cols])

                    for psrc, dstT in ((ps_q0, qT[0]), (ps_q1, qT[1]), (ps_k, kT)):
                        raw = rp.tile([128, CH], F32R, tag="qraw")
                        nc.vector.tensor_copy(raw, psrc)
                        ps_sw = trps.tile([128, CH], F32, tag="tr")
                        nc.tensor.matmul(ps_sw, pswap, raw, start=True, stop=True)
                        t2 = rp.tile([128, CH], F32, tag="t2")
                        nc.vector.tensor_tensor(
                            t2, raw.bitcast(F32), cos_t, op=mybir.AluOpType.mult
                        )
                        # sw *= sin in place (PSUM), then add -> rope output
                        nc.vector.tensor_tensor(ps_sw, ps_sw, sin_t, op=mybir.AluOpType.mult)
                        nc.vector.tensor_tensor(
                            dstT[:, cols], t2, ps_sw, op=mybir.AluOpType.add
                        )

                    # V: evacuate then PE-transpose to natural [s, d] layout
                    vtmp = rp.tile([128, CH], F32, tag="qraw2")
                    nc.vector.tensor_copy(vtmp, ps_v)
                    ps_vt = trps.tile([128, CH], F32, tag="tr")
                    for u in range(4):
                        nc.tensor.transpose(
                            ps_vt[:, u * 128:(u + 1) * 128],
                            vtmp[:, u * 128:(u + 1) * 128],
                            ident,
                        )
                    nc.vector.tensor_copy(
                        vN[:, 4 * m:4 * m + 4, :],
                        ps_vt.rearrange("p (u d) -> p u d", u=4),
                    )

                    # partial block sums for this chunk's 4 key blocks
                    nc.vector.tensor_reduce(
                        bm[:, 4 * m:4 * m + 4],
                        kT.bitcast(F32)[:, cols].rearrange("p (b t) -> p b t", b=4),
                        axis=mybir.AxisListType.X,
                        op=mybir.AluOpType.add,
                    )
                    # gating flags for this chunk's query blocks (needs bm 0..i)
                    if m >= 2:
                        for h in range(HPC):
                            for i in range(4 * m, 4 * m + 4):
                                nbk = 4 * m + 4  # even N; cols > i unused
                                ps_g = trps.tile([128, NB], F32, tag="g", bufs=1)
                                nc.tensor.matmul(
                                    ps_g[:, 0:nbk],
                                    qT[h][:, i * 128:(i + 1) * 128],
                                    bm[:, 0:nbk],
                                    start=True,
                                    stop=True,
                                )
                                cmp = gp.tile([128, NB], F32, tag="cmp", bufs=2)
                                cnt = gp.tile([128, 1], F32, tag="cnt", bufs=2)
                                nc.vector.tensor_scalar(
                                    out=cmp[:, 0:i],
                                    in0=ps_g[:, 0:i],
                                    scalar1=ps_g[:, i:i + 1],
                                    scalar2=None,
                                    op0=mybir.AluOpType.is_gt,
                                )
                                nc.vector.tensor_reduce(
                                    cnt,
                                    cmp[:, 0:i],
                                    axis=mybir.AxisListType.X,
                                    op=mybir.AluOpType.add,
                                )
                                # notflag: 1.0 -> own block selected (keep all)
                                nf = gp.tile([128, 1], F32, tag="nf", bufs=2)
                                nc.vector.tensor_scalar(
                                    out=nf,
                                    in0=cnt,
                                    scalar1=float(TOPK) - 0.5,
                                    scalar2=None,
                                    op0=mybir.AluOpType.is_lt,
                                )
                                ps_ft = trps.tile([1, 128], F32, tag="ft", bufs=1)
                                nc.tensor.transpose(ps_ft, nf, ident)
                                nc.vector.tensor_copy(
                                    Ft[h][:, (i - TOPK) * 128:(i - TOPK + 1) * 128],
                                    ps_ft,
                                )

            # ---------------- phases 3+4 ------------------------------------
            _phase34(nc, tc, qT, kT, vN, trik4, ones_c, ones_cb, ones_r, Ft,
                     wo_sb, out_d)

    nc.compile()
    _CACHE[key] = nc
    return nc


def _phase34(nc, tc, qT, kT, vN, trik4, ones_c, ones_cb, ones_r, Ft, wo_sb, out_d):
    # ------- phase 3: attention with interleaved output projection -------
    # (wo(m) right after attn(m) so the output DMA spreads over the whole
    # kernel instead of piling into a DMA-bound tail phase)
    with (
        tc.tile_pool(name="att", bufs=8) as ap,
        tc.tile_pool(name="attb", bufs=2) as ab,
        tc.tile_pool(name="dtree", bufs=6) as dtp,
        tc.tile_pool(name="oTs", bufs=4) as otp,
        tc.tile_pool(name="outp", bufs=6) as outp,
        tc.tile_pool(name="att_s", bufs=2, space="PSUM") as pss,
        tc.tile_pool(name="att_o", bufs=3, space="PSUM") as pso,
        tc.tile_pool(name="att_d", bufs=1, space="PSUM") as psd,
    ):
        # wo(m-1) work items are interleaved between chunk m's attention
        # pairs: the PE executes in program order, so without this the PE
        # sits idle ~300 ns per pair waiting on the ACT exp, and the wo
        # matmuls pile into a serial section at each chunk boundary.
        def emit_one_wo(oTc_mm, st, n):
            ncols = slice(n * 512, (n + 1) * 512)
            ps_w = pso.tile([128, 512], F32, tag="o", name="psw")
            act_evac = (st * 4 + n) % 2 == 0
            nc.tensor.matmul(
                ps_w,
                oTc_mm[0][:, (st % 4) * 128:(st % 4 + 1) * 128],
                wo_sb[:, 0, ncols],
                start=True,
                stop=False,
            )
            nc.tensor.matmul(
                ps_w,
                oTc_mm[1][:, (st % 4) * 128:(st % 4 + 1) * 128],
                wo_sb[:, 1, ncols],
                start=False,
                stop=True,
            )
            osb = outp.tile([128, 512], F16, tag="ow")
            # mid-chunk items must NOT evacuate on ACT: the in-order ACT
            # stream is the attention pacer (exp), a copy there stalls it
            if act_evac:
                nc.scalar.copy(osb, ps_w)
                nc.sync.dma_start(
                    out=out_d[st * 128:(st + 1) * 128, ncols], in_=osb
                )
            else:
                nc.vector.tensor_copy(osb, ps_w)
                nc.gpsimd.dma_start(
                    out=out_d[st * 128:(st + 1) * 128, ncols], in_=osb
                )

        prev_wo = None  # (m-1, its oTc tiles); items drain during chunk m
        for m in range(NCHUNK):
            nsk = 8 if m < 2 else 4 * m + 4
            cols = slice(m * CH, (m + 1) * CH)
            ps_o = [pso.tile([128, CH], F32, tag="o", name=f"o{h}") for h in range(HPC)]
            oTc = [
                otp.tile([128, CH], F16, tag="oTc", name=f"oTc{h}")
                for h in range(HPC)
            ]
            # precompute the 4 diagonal-mask slabs for this chunk's band in
            # one broadcast matmul + one DVE max per head (off the exp->PV
            # critical path)
            mk4 = {}
            if m >= 2:
                for h in range(HPC):
                    ps_bc = pss.tile([128, 2, CH], F32, tag="s", name="bc")
                    nc.tensor.matmul(
                        ps_bc[:, 0, :],
                        ones_r,
                        Ft[h][:, (4 * m - TOPK) * 128:(4 * m - TOPK + 4) * 128],
                        start=True,
                        stop=True,
                    )
                    mk = ab.tile([128, CH], BF16, tag="mk4", bufs=2)
                    nc.vector.tensor_tensor(
                        mk, trik4, ps_bc[:, 0, :], op=mybir.AluOpType.max
                    )
                    mk4[h] = mk
            # one PSUM bank holds both heads' denominator rows (matmul output
            # base partition must be 0/32/64)
            ps_den2 = psd.tile([64, CH], F32, tag="den", name="den")
            ps_den = [ps_den2[32 * h:32 * h + 1, :] for h in range(HPC)]
            pexps = {}
            for jp in range(0, nsk, 2):
                for h in range(HPC):
                    ps_s2 = pss.tile([128, 2, CH], F32, tag="s", bufs=2)
                    for u in (0, 1):
                        j = jp + u
                        band = m >= 2 and j >= 4 * m
                        col0 = (j - 4 * m) * 128 if band else 0
                        nc.tensor.matmul(
                            ps_s2[:, u, col0:],
                            kT[:, j * 128:(j + 1) * 128],
                            qT[h][:, m * CH + col0:(m + 1) * CH],
                            start=True,
                            stop=True,
                        )
                    # one exp covers the j-pair, trimmed to the band pair's
                    # valid rectangle; remaining stale-PSUM margin (the later
                    # band block's left edge) exps to junk no consumer reads
                    ec0 = (jp - 4 * m) * 128 if (m >= 2 and jp >= 4 * m) else 0
                    pexp2 = ap.tile([128, 2, CH], BF16, tag="pexp", bufs=8)
                    nc.scalar.activation(
                        out=pexp2[:, :, ec0:],
                        in_=ps_s2[:, :, ec0:],
                        func=mybir.ActivationFunctionType.Exp,
                        scale=INV_SQRT_D,
                    )
                    for u in (0, 1):
                        j = jp + u
                        band = m >= 2 and j >= 4 * m
                        col0 = (j - 4 * m) * 128 if band else 0
                        pexp = pexp2[:, u, :]
                        if band:
                            nc.vector.tensor_tensor(
                                pexp[:, col0:col0 + 128],
                                pexp[:, col0:col0 + 128],
                                mk4[h][:, col0:col0 + 128],
                                op=mybir.AluOpType.mult,
                            )
                            # band: per-j denominator on the valid columns
                            nc.tensor.matmul(
                                ps_den[h][:, col0:],
                                ones_cb,
                                pexp[:, col0:],
                                start=False,
                                stop=(j == nsk - 1),
                            )
                        else:
                            pexps[(j, h)] = pexp
                        nc.tensor.matmul(
                            ps_o[h][:, col0:],
                            vN[:, j, :],
                            pexp[:, col0:],
                            start=(j == 0),
                            stop=(j == nsk - 1),
                        )
                # dense quad complete: bf16 tree-sum on DVE, one den matmul
                j = jp + 1
                if (jp + 2) % 4 == 0 and not (m >= 2 and j >= 4 * m):
                    jq = jp - 2
                    for h in range(HPC):
                        t01 = dtp.tile([128, CH], BF16, tag="t01")
                        t23 = dtp.tile([128, CH], BF16, tag="t23")
                        nc.vector.tensor_tensor(
                            t01, pexps.pop((jq, h)), pexps.pop((jq + 1, h)),
                            op=mybir.AluOpType.add,
                        )
                        nc.vector.tensor_tensor(
                            t23, pexps.pop((jq + 2, h)), pexps.pop((jq + 3, h)),
                            op=mybir.AluOpType.add,
                        )
                        nc.vector.tensor_tensor(t01, t01, t23, op=mybir.AluOpType.add)
                        nc.tensor.matmul(
                            ps_den[h],
                            ones_cb,
                            t01,
                            start=(jq == 0),
                            stop=(m < 2 and j == nsk - 1),
                        )
            for h in range(HPC):
                rec = ab.tile([1, CH], F32R, tag="rec")
                nc.vector.reciprocal(rec, ps_den[h])
                ps_rb = pss.tile([128, 2, CH], F32, tag="s", name="rb")
                nc.tensor.matmul(ps_rb[:, 0, :], ones_r, rec, start=True, stop=True)
                bc_sb = ab.tile([128, CH], F32, tag="bcs")
                nc.scalar.copy(bc_sb, ps_rb[:, 0, :])
                nc.vector.tensor_tensor(
                    oTc[h], ps_o[h], bc_sb, op=mybir.AluOpType.mult
                )
            if prev_wo is not None:
                for u in range(4):
                    for n in range(4):
                        emit_one_wo(prev_wo[1], 4 * prev_wo[0] + u, n)
            prev_wo = (m, oTc)
        for u in range(4):
            for n in range(4):
                emit_one_wo(prev_wo[1], 4 * prev_wo[0] + u, n)
def kernel(x, wq, wk, wv, wo):
    bs = np.asarray(x).shape[0]
    in_maps = make_in_maps(x, wq, wk, wv, wo)
    nc = _build_nc()
    res = run_bass_kernel_spmd(nc, in_maps, list(range(N_CORES)))
    out = res.results[0]["out"].astype(np.float64)
    for c in range(1, N_CORES):
        out += res.results[c]["out"]
    return out.astype(np.float32).reshape(bs, SEQ, DIM)


if __name__ == "__main__":
    rng = np.random.default_rng(0)
    xs = {
        "x": rng.standard_normal((1, SEQ, DIM), dtype=np.float32),
        "wq": rng.standard_normal((DIM, DIM), dtype=np.float32) * (DIM ** -0.5),
        "wk": rng.standard_normal((DIM, DIM // 2), dtype=np.float32) * (DIM ** -0.5),
        "wv": rng.standard_normal((DIM, DIM // 2), dtype=np.float32) * (DIM ** -0.5),
        "wo": rng.standard_normal((DIM, DIM), dtype=np.float32) * (DIM ** -0.5),
    }
    out = kernel(**xs)
    print("out", out.shape, out.dtype, np.abs(out).max())

